# revision 56
# baseline (speedup 1.0000x reference)
"""Multi-head attention (b=2, n=2048, dim=1024, h=16, dh=64) on 8 TRN2 NeuronCores.

Sharding: 32 (batch, head) pairs -> 8 cores x (1 batch, 4 heads). No collectives.
Per core (all device inputs host-packed to match the SBUF layouts so every
DMA is a contiguous 2-8KB-per-partition-run transfer):
  inputs : xT  [128, 4*8*512] bf16  x[b].T packed n-chunk-major:
                                    element (p, nch, kt, i) = x[b].T[kt*128+p, nch*512+i]
           wq  [128, 2*8*128] bf16  w_qkv q-columns for this core's 4 heads,
                                    pre-scaled by 1/8, packed
                                    (p, hp, kt, c) = w[kt*128+p, hp*128+c]
           wk  [128, 2*8*128] bf16  (same packing)
           wv  [128, 2*8*128] bf16  (same packing)
  output : out [4*65, 2048] bf16  (per local head: rows 0-63 = unnormalized (attn@v)^T,
                                   row 64 = softmax denominator per query)
Host divides by the denominator and transposes back to [b, n, h*dh].

Device pipeline per core:
  qT/kT = (w.T @ x.T) in [d, n] layout, head-pairs packed 2x64 on partitions (bf16)
  V     = (x @ wv)    in [n, d] layout with a ones column appended (bf16)
  per head pair, per 512-wide query chunk, per 128-wide key block:
    S^T[j,i] = kT.T @ qT   (two K=64 matmuls packed into PE row-groups 0-63 / 64-127)
    A^T      = exp(S^T)    (one ACT instr over both heads' PSUM banks, f32 -> bf16)
    O^T     += [V|1].T @ A^T  (PSUM-accumulated over key blocks; row 64 = rowsum)

Startup: dummy matmuls warm the PE clock gate during the DMA window, the
xT transfer is staged by n-chunk so the first projections start ~10us in,
and the first scores wait only on a narrow kT j-block-0 chain.  The
remaining projections weave into the attention periods' PE slack, paced
under the ACT exp cadence with per-chunk deadlines.
"""

import numpy as np
import ml_dtypes

B, N, DIM = 2, 2048, 1024
HEADS, DH = 16, 64
P = 128
KT = DIM // P          # 8 k-tiles
NT = N // P            # 16 n/j blocks
NCH = N // 512         # 4 chunks of 512
HL = 4                 # local heads per core
OROWS = HL * (DH + 1)  # 260 output rows per core

_CACHE = {}
LAST_RESULTS = None
TRACE = False


def _build_nc():
    from contextlib import ExitStack

    import concourse.bass as bass
    import concourse.tile as tile
    from concourse import bacc, mybir

    bf16 = mybir.dt.bfloat16
    f32 = mybir.dt.float32

    nc = bacc.Bacc("TRN2", target_bir_lowering=False)

    xT_d = nc.dram_tensor("xT", [P, KT * N], bf16, kind="ExternalInput")
    # weights host-packed to the SBUF layout [p, hp, kt, c] so the DMAs are
    # flat 2D with 2KB per-partition runs and can be hp-sliced
    wq_d = nc.dram_tensor("wq", [P, 2 * KT * P], bf16, kind="ExternalInput")
    wk_d = nc.dram_tensor("wk", [P, 2 * KT * P], bf16, kind="ExternalInput")
    wv_d = nc.dram_tensor("wv", [P, 2 * KT * P], bf16, kind="ExternalInput")
    out_d = nc.dram_tensor("out", [OROWS, N], bf16, kind="ExternalOutput")

    # out rows viewed as [row-within-head, head, n] for packed output DMAs
    out_r = out_d[:, :].rearrange("(hh r) n -> r hh n", r=DH + 1)
    xT_r = xT_d[:, :].rearrange("p (nch kt n) -> p nch kt n", nch=NCH, kt=KT)

    with tile.TileContext(nc) as tc, ExitStack() as ctx:
        sing = ctx.enter_context(tc.tile_pool(name="sing", bufs=1))
        spool = ctx.enter_context(
            tc.tile_pool(name="s_ps", bufs=3, space=bass.MemorySpace.PSUM)
        )
        opool = ctx.enter_context(
            tc.tile_pool(name="o_ps", bufs=1, space=bass.MemorySpace.PSUM)
        )
        apool = ctx.enter_context(tc.tile_pool(name="a_sb", bufs=16))
        copool = ctx.enter_context(tc.tile_pool(name="o_sb", bufs=4))

        # persistent SBUF tensors
        xT = sing.tile([P, NCH, KT, 512], bf16, tag="xT")
        wq = sing.tile([P, 2, KT, P], bf16, tag="wq")
        wk = sing.tile([P, 2, KT, P], bf16, tag="wk")
        wv = sing.tile([P, 2, KT, P], bf16, tag="wv")
        # head-pair packed projections: partitions 0-63 head A dims, 64-127 head B
        qT = [sing.tile([P, N], bf16, tag=f"qT{i}", name=f"qT{i}") for i in range(2)]
        kT = [sing.tile([P, N], bf16, tag=f"kT{i}", name=f"kT{i}") for i in range(2)]
        # V in [j, d] layout per j-block per head, with ones column at d=64
        v = sing.tile([P, NT, HL, DH + 1], bf16, tag="v")
        # tiny zero tile fueling the PE warm-up matmuls
        warm = apool.tile([P, 640], bf16, tag="warm", bufs=1)

        # a dummy-matmul run keeps the PE busy through the HAM activity
        # window while the first input DMAs land so real work starts at
        # 2.4 GHz; wide N=512 matmuls register as sustained busy (N=64
        # dummies measurably do not flip the clock gate)
        nc.vector.memset(warm[:], 0.0)
        wps = spool.tile([P, 1024], f32, tag="sp", name="wps")
        for _ in range(8):
            nc.tensor.matmul(
                wps[0:64, 0:512], warm[:, 0:64], warm[:, 128:640],
                start=True, stop=True,
            )

        # input DMAs: k/q weights on the software ring (small, needed first);
        # xT staged by (n-chunk, k-tile) across both HWDGE rings so the
        # chunk-0 projections can start ~6us in instead of after the full
        # 4MB transfer; wv halves ride the HW rings right after chunk 0,
        # in time for the woven V units.
        hpw = lambda t, h: t[:, h * 1024 : (h + 1) * 1024].rearrange(
            "p (kt c) -> p kt c", kt=KT
        )
        # hp0 k/q weights on the software ring (small, needed first); xT in
        # two n-stages of kt-quads on the HW rings (2KB runs, few triggers —
        # each trigger is a real instruction on its engine's queue, and the
        # ACT queue must stay clear for the exps); wv and the hp1 weights
        # ride the HW rings between the stages so stage A isn't contended.
        nc.gpsimd.dma_start(out=wk[:, 0], in_=hpw(wk_d, 0))
        nc.gpsimd.dma_start(out=wq[:, 0], in_=hpw(wq_d, 0))
        # chunk 0 as four kt-pair transfers, ring-assigned to match the
        # chains' sequential k-tile order (sync gets kt0-3, the ~3us-lagged
        # scalar ring gets kt4-7) so no single PE stall crosses the HAM
        # re-throttle window
        nc.sync.dma_start(out=xT[:, 0, 0:2, :], in_=xT_r[:, 0, 0:2, :])
        nc.sync.dma_start(out=xT[:, 0, 2:4, :], in_=xT_r[:, 0, 2:4, :])
        nc.scalar.dma_start(out=xT[:, 0, 4:6, :], in_=xT_r[:, 0, 4:6, :])
        nc.scalar.dma_start(out=xT[:, 0, 6:8, :], in_=xT_r[:, 0, 6:8, :])
        for nch in range(NCH):
            if nch > 0:
                nc.sync.dma_start(out=xT[:, nch, 0:4, :], in_=xT_r[:, nch, 0:4, :])
                nc.scalar.dma_start(out=xT[:, nch, 4:8, :], in_=xT_r[:, nch, 4:8, :])
            if nch == 0:
                nc.sync.dma_start(out=wv[:, 0], in_=hpw(wv_d, 0))
                nc.scalar.dma_start(out=wv[:, 1], in_=hpw(wv_d, 1))
                # hp1 weights behind wv on the HW rings: off the software
                # ring so they don't contend with chunk 0 before the first
                # exp (they're read from ~p24)
                nc.sync.dma_start(out=wk[:, 1], in_=hpw(wk_d, 1))
                nc.scalar.dma_start(out=wq[:, 1], in_=hpw(wq_d, 1))

        # ---- projections ----
        # k, q: out[c, n] = w[:, c].T @ xT.  hp0-chunk0 upfront; hp1 woven
        # into attention-hp0's periods (PE fills slack while ACT runs exp).
        # The earliest chains borrow the (still idle) PV-accumulator PSUM
        # slots so the 3-deep spool rotation is left to the scores/exp
        # pipeline — otherwise the first exps serialize against chain psums.
        def proj_unit(wt, dst, hp, nch, pool_tag=None):
            """Emit the 8 K-accumulated matmuls + copy for one 512-col chunk,
            returned as two 4-matmul halves so weaving stays fine-grained."""
            state = {}

            def half(h):
                if h == 0:
                    if pool_tag is None:
                        state["ps"] = spool.tile([P, 512], f32, tag="sp", name="ps")
                    else:
                        state["ps"] = opool.tile(
                            [P, 512], f32, tag=pool_tag, name="ps"
                        )
                ps = state["ps"]
                for kt in range(4 * h, 4 * h + 4):
                    nc.tensor.matmul(
                        ps[:],
                        wt[:, hp, kt, :],
                        xT[:, nch, kt, :],
                        start=(kt == 0),
                        stop=(kt == KT - 1),
                    )
                if h == 1:
                    nc.vector.tensor_copy(dst[:, nch * 512 : (nch + 1) * 512], ps[:])

            return [lambda: half(0), lambda: half(1)]

        # ones column of V (softmax denominator comes out of the PV matmul)
        nc.vector.memset(v[:, :, :, DH : DH + 1], 1.0)

        def upfront_chains():
            """k j-block 0 and q chunk 0 interleaved per k-tile (so both
            chains consume the staged xT slices as they arrive), letting the
            first scores matmul fire right after the last slice lands; the
            rest of the kT chunk follows."""
            psk = opool.tile([P, P], f32, tag="oA", name="psk")
            psq = opool.tile([P, 512], f32, tag="oB", name="psq")
            for kt in range(KT):
                nc.tensor.matmul(
                    psk[:], wk[:, 0, kt, :], xT[:, 0, kt, 0:P],
                    start=(kt == 0), stop=(kt == KT - 1),
                )
                nc.tensor.matmul(
                    psq[:], wq[:, 0, kt, :], xT[:, 0, kt, :],
                    start=(kt == 0), stop=(kt == KT - 1),
                )
            nc.vector.tensor_copy(kT[0][:, 0:P], psk[:])
            nc.vector.tensor_copy(qT[0][:, 0:512], psq[:])

        def upfront_krest(n0, n1):
            ps = opool.tile([P, n1 - n0], f32, tag="oA", name="pskr")
            for kt in range(KT):
                nc.tensor.matmul(
                    ps[:], wk[:, 0, kt, :], xT[:, 0, kt, n0:n1],
                    start=(kt == 0), stop=(kt == KT - 1),
                )
            nc.vector.tensor_copy(kT[0][:, n0:n1], ps[:])

        # remaining projections are woven into the attention periods; each
        # woven chunk lands (in emission order) before the first scores
        # matmul that reads it.  The kT0 chunks 1-3 lead the weave (instead
        # of running upfront) so the first exps don't wait on the later
        # staged xT chunks.
        def full_unit(halves):
            return lambda: [h() for h in halves]

        woven = [
            full_unit(proj_unit(wk, kT[0], 0, 1, pool_tag="oA")),
            full_unit(proj_unit(wk, kT[0], 0, 2, pool_tag="oB")),
            full_unit(proj_unit(wk, kT[0], 0, 3, pool_tag="oA")),
            full_unit(proj_unit(wq, qT[0], 0, 1, pool_tag="oB")),
        ]
        # ... as half-units: a full 2.56us unit in one period outruns the
        # 2-period exp lookahead and stalls ACT; consecutive-period halves
        # stay under it (and keep the chain psum's spool slot turning over)
        woven_rest = []
        for nch in range(2, NCH):
            woven_rest.extend(proj_unit(wq, qT[0], 0, nch))
        for wt, dst in ((wk, kT[1]), (wq, qT[1])):
            for nch in range(NCH):
                woven_rest.extend(proj_unit(wt, dst, 1, nch))

        # V: out[n, c] = xT[:, ntile].T @ wv   -> [128 n, 256 c]
        def v_unit(nt):
            state = {}

            def half(h):
                if h == 0:
                    state["ps"] = spool.tile([P, HL * DH], f32, tag="sp", name="psv")
                ps = state["ps"]
                for kt in range(4 * h, 4 * h + 4):
                    nc.tensor.matmul(
                        ps[:],
                        xT[:, nt // 4, kt, (nt % 4) * P : (nt % 4 + 1) * P],
                        wv[:, :, kt, :],
                        start=(kt == 0),
                        stop=(kt == KT - 1),
                    )
                if h == 1:
                    # scatter the 4 heads' 64 cols into the [NT, HL, 65] layout
                    nc.vector.tensor_copy(
                        v[:, nt, :, 0:DH],
                        ps[:].rearrange("p (h d) -> p h d", h=HL),
                    )

            return [lambda: half(0), lambda: half(1)]

        v_units = [full_unit(v_unit(nt)) for nt in range(NT)]

        # ---- attention ----
        # 8 blocks of 16 periods (one per (hp, ic)).  ACT runs one
        # [128, 1024] exp per period back-to-back; PE emits scores two
        # periods ahead (spool rotation) plus woven projection work; PV runs
        # as dense 8-matmul bursts every 4 periods (no exp-latency exposure).
        # Block 0 weaves the V projection (PV bursts shifted late until V is
        # ready); blocks 1+ weave the remaining q/k projections.
        blocks = [(hp, ic) for hp in range(2) for ic in range(NCH)]
        ats = {}
        opairs = {}
        sp_ahead = {}

        def emit_scores(b, jb):
            hp, ic = blocks[b]
            i0, j0 = ic * 512, jb * P
            sp = spool.tile([P, 1024], f32, tag="sp", name="sp")
            nc.tensor.matmul(
                sp[:, 0:512],
                kT[hp][0:DH, j0 : j0 + P],
                qT[hp][0:DH, i0 : i0 + 512],
                start=True, stop=True, tile_position=(0, 0),
            )
            nc.tensor.matmul(
                sp[:, 512:1024],
                kT[hp][DH:P, j0 : j0 + P],
                qT[hp][DH:P, i0 : i0 + 512],
                start=True, stop=True, tile_position=(64, 0),
            )
            return sp

        def emit_exp(b, jb, sp):
            at = apool.tile([P, 1024], bf16, tag="at", name="at")
            nc.scalar.activation(at[:], sp[:], mybir.ActivationFunctionType.Exp)
            ats[(b, jb)] = at

        def fetch_scores(b, jb):
            key = (b, jb)
            if key in sp_ahead:
                return sp_ahead.pop(key)
            return emit_scores(b, jb)

        def emit_pv(b, jbs, writeback=False):
            """PV matmuls of block b for the given j-blocks (dense burst);
            with writeback, each head's copy+DMA follows its last matmul so
            it overlaps the other head's PV."""
            hp, ic = blocks[b]
            if b not in opairs:
                opairs[b] = (
                    opool.tile([DH + 1, 512], f32, tag="oA", name="oA"),
                    opool.tile([DH + 1, 512], f32, tag="oB", name="oB"),
                )
            oA, oB = opairs[b]
            i0 = ic * 512
            for col, o in ((0, oA), (1, oB)):
                for jb in jbs:
                    nc.tensor.matmul(
                        o[:],
                        v[:, jb, 2 * hp + col, :],
                        ats[(b, jb)][:, 512 * col : 512 * col + 512],
                        start=(jb == 0), stop=(jb == NT - 1),
                    )
                if writeback:
                    os = copool.tile([DH + 1, 1, 512], bf16, tag="os", name="os")
                    nc.vector.tensor_copy(os[:, 0, :], o[:])
                    nc.sync.dma_start(
                        out=out_r[:, 2 * hp + col : 2 * hp + col + 1, i0 : i0 + 512],
                        in_=os[:],
                    )
            for jb in jbs:
                del ats[(b, jb)]

        def emit_pv_quarter(b, q):
            emit_pv(b, list(range(4 * q, 4 * q + 4)), writeback=(q == 3))

        LA = 2  # scores lookahead depth
        nblocks = len(blocks)
        # prime the pipeline: scores(0,0) right after the interleaved narrow
        # chains so the first exp starts as soon as chunk 0 has landed
        upfront_chains()
        sp_ahead[(0, 0)] = emit_scores(0, 0)
        upfront_krest(P, 2 * P)
        sp_ahead[(0, 1)] = emit_scores(0, 1)
        upfront_krest(2 * P, 512)
        for b in range(nblocks):
            for jb in range(NT):
                emit_exp(b, jb, fetch_scores(b, jb))
                la = jb + LA
                if la < NT:
                    if (b, la) not in sp_ahead:
                        sp_ahead[(b, la)] = emit_scores(b, la)
                elif b + 1 < nblocks:
                    sp_ahead[(b + 1, la - NT)] = emit_scores(b + 1, la - NT)
                if jb == NT - 1 and b + 1 < nblocks:
                    # boundary prefetch into the idle third spool slot: gives
                    # ACT a 3rd exp of cover across the 16-matmul PV burst
                    sp_ahead[(b + 1, LA)] = emit_scores(b + 1, LA)
                # woven PE filler: 1 unit/period in block 0 plus 2 V units
                # once the k/q chunks are done; then 1 unit per 6 periods,
                # with the last two q-hp1 chunks deferred to blocks 5-6
                # where the PE has slack
                p = b * NT + jb
                if b == 0:
                    if woven:
                        woven.pop(0)()
                    for _ in range(2):
                        if not woven and v_units:
                            v_units.pop(0)()
                elif woven_rest and (p - NT) % 6 in (5, 0) and 20 < p < 65:
                    woven_rest.pop(0)()
                elif woven_rest and p in (80, 81, 88, 89):
                    woven_rest.pop(0)()
                # PV bursts (block 0's deferred until the woven V is ready);
                # each block's final quarter is split into two half-bursts so
                # the block boundary doesn't pile 8 matmuls after the last exp
                if b == 0:
                    if jb == 8:
                        emit_pv_quarter(0, 0)
                    elif jb == 10:
                        emit_pv_quarter(0, 1)
                    elif jb == 12:
                        emit_pv_quarter(0, 2)
                elif jb in (4, 8, 12):
                    emit_pv_quarter(b, jb // 4 - 1)
                if jb == NT - 2:
                    emit_pv(b, [12, 13, 14])
                elif jb == NT - 1:
                    while v_units:
                        v_units.pop(0)()
                    emit_pv(b, [15], writeback=True)

    nc.compile()
    return nc


def _get_nc():
    if "nc" not in _CACHE:
        _CACHE["nc"] = _build_nc()
    return _CACHE["nc"]


def _prepare_in_maps(x, w_qkv):
    bf = ml_dtypes.bfloat16
    x = np.asarray(x, dtype=np.float32)
    w = np.asarray(w_qkv, dtype=np.float32)
    scale = DH ** -0.5
    in_maps = []
    xT_b = [
        np.ascontiguousarray(
            x[b].T.reshape(KT, P, NCH, 512).transpose(1, 2, 0, 3).reshape(P, KT * N)
        ).astype(bf)
        for b in range(B)
    ]
    def pack_w(w_slice):
        # [1024, 256] -> [p, hp, kt, c] -> [128, 2048]
        t = w_slice.reshape(KT, P, 2, P).transpose(1, 2, 0, 3).reshape(P, 2 * KT * P)
        return np.ascontiguousarray(t).astype(bf)

    for c in range(8):
        b, hg = divmod(c, 4)
        cs = slice(hg * HL * DH, (hg + 1) * HL * DH)
        in_maps.append(
            {
                "xT": xT_b[b],
                "wq": pack_w(w[:, cs] * scale),
                "wk": pack_w(w[:, 1024:2048][:, cs]),
                "wv": pack_w(w[:, 2048:3072][:, cs]),
            }
        )
    return in_maps


def _assemble(outs):
    full = np.empty((B, N, HEADS * DH), dtype=np.float32)
    for c in range(8):
        b, hg = divmod(c, 4)
        o = np.asarray(outs[c], dtype=np.float32).reshape(HL, DH + 1, N)
        norm = o[:, :DH, :] / o[:, DH : DH + 1, :]  # [hl, d, n]
        full[b, :, hg * HL * DH : (hg + 1) * HL * DH] = norm.transpose(2, 0, 1).reshape(
            N, HL * DH
        )
    return full


def kernel(x, w_qkv):
    global LAST_RESULTS
    from concourse.bass_utils import run_bass_kernel_spmd

    nc = _get_nc()
    in_maps = _prepare_in_maps(x, w_qkv)
    last_err = None
    for _ in range(3):  # the runtime occasionally throws a transient device error
        try:
            res = run_bass_kernel_spmd(
                nc,
                in_maps,
                core_ids=list(range(8)),
                trace=TRACE,
                trace_cores=[0] if TRACE else None,
            )
            break
        except Exception as e:
            last_err = e
    else:
        raise last_err
    LAST_RESULTS = res
    return _assemble([r["out"] for r in res.results])



# revision 57
# speedup vs baseline: 1.1932x; 1.1932x over previous
"""Multi-head attention (b=2, n=2048, dim=1024, h=16, dh=64) on 8 TRN2 NeuronCores.

Sharding: 32 (batch, head) pairs -> 8 cores x (1 batch, 4 heads). No collectives.
Per core (all device inputs host-packed to match the SBUF layouts so every
DMA is a contiguous 2-8KB-per-partition-run transfer):
  inputs : xT  [128, 4*8*512] bf16  x[b].T packed n-chunk-major:
                                    element (p, nch, kt, i) = x[b].T[kt*128+p, nch*512+i]
           wq  [128, 2*8*128] bf16  w_qkv q-columns for this core's 4 heads,
                                    pre-scaled by 1/8, packed
                                    (p, hp, kt, c) = w[kt*128+p, hp*128+c]
           wk  [128, 2*8*128] bf16  (same packing)
           wv  [128, 2*8*128] bf16  (same packing)
  output : out [4*65, 2048] bf16  (per local head: rows 0-63 = unnormalized (attn@v)^T,
                                   row 64 = softmax denominator per query)
Host divides by the denominator and transposes back to [b, n, h*dh].

Device pipeline per core:
  qT/kT = (w.T @ x.T) in [d, n] layout, head-pairs packed 2x64 on partitions (bf16)
  V     = (x @ wv)    in [n, d] layout with a ones column appended (bf16)
  per head pair, per 512-wide query chunk, per 128-wide key block:
    S^T[j,i] = kT.T @ qT   (two K=64 matmuls packed into PE row-groups 0-63 / 64-127)
    A^T      = exp(S^T)    (one ACT instr over both heads' PSUM banks, f32 -> bf16)
    O^T     += [V|1].T @ A^T  (PSUM-accumulated over key blocks; row 64 = rowsum)

Startup: dummy matmuls warm the PE clock gate during the DMA window, the
xT transfer is staged by n-chunk so the first projections start ~10us in,
and the first scores wait only on a narrow kT j-block-0 chain.  The
remaining projections weave into the attention periods' PE slack, paced
under the ACT exp cadence with per-chunk deadlines.
"""

import numpy as np
import ml_dtypes

B, N, DIM = 2, 2048, 1024
HEADS, DH = 16, 64
P = 128
KT = DIM // P          # 8 k-tiles
NT = N // P            # 16 n/j blocks
NCH = N // 512         # 4 chunks of 512
HL = 4                 # local heads per core
OROWS = HL * (DH + 1)  # 260 output rows per core

_CACHE = {}
LAST_RESULTS = None
TRACE = False


def _build_nc():
    from contextlib import ExitStack

    import concourse.bass as bass
    import concourse.tile as tile
    from concourse import bacc, mybir

    bf16 = mybir.dt.bfloat16
    f32 = mybir.dt.float32

    nc = bacc.Bacc("TRN2", target_bir_lowering=False)

    xT_d = nc.dram_tensor("xT", [P, KT * N], bf16, kind="ExternalInput")
    # weights host-packed to the SBUF layout [p, hp, kt, c] so the DMAs are
    # flat 2D with 2KB per-partition runs and can be hp-sliced
    wq_d = nc.dram_tensor("wq", [P, 2 * KT * P], bf16, kind="ExternalInput")
    wk_d = nc.dram_tensor("wk", [P, 2 * KT * P], bf16, kind="ExternalInput")
    wv_d = nc.dram_tensor("wv", [P, 2 * KT * P], bf16, kind="ExternalInput")
    out_d = nc.dram_tensor("out", [OROWS, N], bf16, kind="ExternalOutput")

    # out rows viewed as [row-within-head, head, n] for packed output DMAs
    out_r = out_d[:, :].rearrange("(hh r) n -> r hh n", r=DH + 1)
    xT_r = xT_d[:, :].rearrange("p (nch kt n) -> p nch kt n", nch=NCH, kt=KT)

    with tile.TileContext(nc) as tc, ExitStack() as ctx:
        sing = ctx.enter_context(tc.tile_pool(name="sing", bufs=1))
        spool = ctx.enter_context(
            tc.tile_pool(name="s_ps", bufs=3, space=bass.MemorySpace.PSUM)
        )
        opool = ctx.enter_context(
            tc.tile_pool(name="o_ps", bufs=1, space=bass.MemorySpace.PSUM)
        )
        apool = ctx.enter_context(tc.tile_pool(name="a_sb", bufs=16))
        copool = ctx.enter_context(tc.tile_pool(name="o_sb", bufs=4))

        # persistent SBUF tensors
        xT = sing.tile([P, NCH, KT, 512], bf16, tag="xT")
        wq = sing.tile([P, 2, KT, P], bf16, tag="wq")
        wk = sing.tile([P, 2, KT, P], bf16, tag="wk")
        wv = sing.tile([P, 2, KT, P], bf16, tag="wv")
        # head-pair packed projections: partitions 0-63 head A dims, 64-127 head B
        qT = [sing.tile([P, N], bf16, tag=f"qT{i}", name=f"qT{i}") for i in range(2)]
        kT = [sing.tile([P, N], bf16, tag=f"kT{i}", name=f"kT{i}") for i in range(2)]
        # V in [j, d] layout per j-block per head, with ones column at d=64
        v = sing.tile([P, NT, HL, DH + 1], bf16, tag="v")
        # tiny zero tile fueling the PE warm-up matmuls
        warm = apool.tile([P, 640], bf16, tag="warm", bufs=1)

        # a dummy-matmul run keeps the PE busy through the HAM activity
        # window while the first input DMAs land so real work starts at
        # 2.4 GHz; wide N=512 matmuls register as sustained busy (N=64
        # dummies measurably do not flip the clock gate)
        nc.vector.memset(warm[:], 0.0)
        wps = spool.tile([P, 1024], f32, tag="sp", name="wps")
        for _ in range(8):
            nc.tensor.matmul(
                wps[0:64, 0:512], warm[:, 0:64], warm[:, 128:640],
                start=True, stop=True,
            )

        # input DMAs: k/q weights on the software ring (small, needed first);
        # xT staged by (n-chunk, k-tile) across both HWDGE rings so the
        # chunk-0 projections can start ~6us in instead of after the full
        # 4MB transfer; wv halves ride the HW rings right after chunk 0,
        # in time for the woven V units.
        hpw = lambda t, h: t[:, h * 1024 : (h + 1) * 1024].rearrange(
            "p (kt c) -> p kt c", kt=KT
        )
        # hp0 k/q weights on the software ring (small, needed first); xT in
        # two n-stages of kt-quads on the HW rings (2KB runs, few triggers —
        # each trigger is a real instruction on its engine's queue, and the
        # ACT queue must stay clear for the exps); wv and the hp1 weights
        # ride the HW rings between the stages so stage A isn't contended.
        nc.gpsimd.dma_start(out=wk[:, 0], in_=hpw(wk_d, 0))
        nc.gpsimd.dma_start(out=wq[:, 0], in_=hpw(wq_d, 0))
        # chunk 0 as four kt-pair transfers alternating rings so the first
        # projection chains consume k-tiles as they land instead of stalling
        # on the slower ring's 4-tile half (consecutive same-ring chunk-0
        # transfers reproducibly trigger a ~215us slow-DMA device mode)
        nc.sync.dma_start(out=xT[:, 0, 0:2, :], in_=xT_r[:, 0, 0:2, :])
        nc.scalar.dma_start(out=xT[:, 0, 2:4, :], in_=xT_r[:, 0, 2:4, :])
        nc.sync.dma_start(out=xT[:, 0, 4:6, :], in_=xT_r[:, 0, 4:6, :])
        nc.scalar.dma_start(out=xT[:, 0, 6:8, :], in_=xT_r[:, 0, 6:8, :])
        for nch in range(NCH):
            if nch > 0:
                nc.sync.dma_start(out=xT[:, nch, 0:4, :], in_=xT_r[:, nch, 0:4, :])
                nc.scalar.dma_start(out=xT[:, nch, 4:8, :], in_=xT_r[:, nch, 4:8, :])
            if nch == 0:
                nc.sync.dma_start(out=wv[:, 0], in_=hpw(wv_d, 0))
                nc.scalar.dma_start(out=wv[:, 1], in_=hpw(wv_d, 1))
                # hp1 weights behind wv on the HW rings: off the software
                # ring so they don't contend with chunk 0 before the first
                # exp (they're read from ~p24)
                nc.sync.dma_start(out=wk[:, 1], in_=hpw(wk_d, 1))
                nc.scalar.dma_start(out=wq[:, 1], in_=hpw(wq_d, 1))

        # ---- projections ----
        # k, q: out[c, n] = w[:, c].T @ xT.  hp0-chunk0 upfront; hp1 woven
        # into attention-hp0's periods (PE fills slack while ACT runs exp).
        # The earliest chains borrow the (still idle) PV-accumulator PSUM
        # slots so the 3-deep spool rotation is left to the scores/exp
        # pipeline — otherwise the first exps serialize against chain psums.
        def proj_unit(wt, dst, hp, nch, pool_tag=None):
            """Emit the 8 K-accumulated matmuls + copy for one 512-col chunk,
            returned as two 4-matmul halves so weaving stays fine-grained."""
            state = {}

            def half(h):
                if h == 0:
                    if pool_tag is None:
                        state["ps"] = spool.tile([P, 512], f32, tag="sp", name="ps")
                    else:
                        state["ps"] = opool.tile(
                            [P, 512], f32, tag=pool_tag, name="ps"
                        )
                ps = state["ps"]
                for kt in range(4 * h, 4 * h + 4):
                    nc.tensor.matmul(
                        ps[:],
                        wt[:, hp, kt, :],
                        xT[:, nch, kt, :],
                        start=(kt == 0),
                        stop=(kt == KT - 1),
                    )
                if h == 1:
                    nc.vector.tensor_copy(dst[:, nch * 512 : (nch + 1) * 512], ps[:])

            return [lambda: half(0), lambda: half(1)]

        # ones column of V (softmax denominator comes out of the PV matmul)
        nc.vector.memset(v[:, :, :, DH : DH + 1], 1.0)

        def upfront_chains():
            """k j-block 0 and q chunk 0 interleaved per k-tile (so both
            chains consume the staged xT slices as they arrive), letting the
            first scores matmul fire right after the last slice lands; the
            rest of the kT chunk follows."""
            psk = opool.tile([P, P], f32, tag="oA", name="psk")
            psq = opool.tile([P, 512], f32, tag="oB", name="psq")
            for kt in range(KT):
                nc.tensor.matmul(
                    psk[:], wk[:, 0, kt, :], xT[:, 0, kt, 0:P],
                    start=(kt == 0), stop=(kt == KT - 1),
                )
                nc.tensor.matmul(
                    psq[:], wq[:, 0, kt, :], xT[:, 0, kt, :],
                    start=(kt == 0), stop=(kt == KT - 1),
                )
            nc.vector.tensor_copy(kT[0][:, 0:P], psk[:])
            nc.vector.tensor_copy(qT[0][:, 0:512], psq[:])

        def upfront_krest(n0, n1):
            ps = opool.tile([P, n1 - n0], f32, tag="oA", name="pskr")
            for kt in range(KT):
                nc.tensor.matmul(
                    ps[:], wk[:, 0, kt, :], xT[:, 0, kt, n0:n1],
                    start=(kt == 0), stop=(kt == KT - 1),
                )
            nc.vector.tensor_copy(kT[0][:, n0:n1], ps[:])

        # remaining projections are woven into the attention periods; each
        # woven chunk lands (in emission order) before the first scores
        # matmul that reads it.  The kT0 chunks 1-3 lead the weave (instead
        # of running upfront) so the first exps don't wait on the later
        # staged xT chunks.
        def full_unit(halves):
            return lambda: [h() for h in halves]

        woven = [
            full_unit(proj_unit(wk, kT[0], 0, 1, pool_tag="oA")),
            full_unit(proj_unit(wk, kT[0], 0, 2, pool_tag="oB")),
            full_unit(proj_unit(wk, kT[0], 0, 3, pool_tag="oA")),
            full_unit(proj_unit(wq, qT[0], 0, 1, pool_tag="oB")),
        ]
        # ... as half-units: a full 2.56us unit in one period outruns the
        # 2-period exp lookahead and stalls ACT; consecutive-period halves
        # stay under it (and keep the chain psum's spool slot turning over)
        woven_rest = []
        for nch in range(2, NCH):
            woven_rest.extend(proj_unit(wq, qT[0], 0, nch))
        for wt, dst in ((wk, kT[1]), (wq, qT[1])):
            for nch in range(NCH):
                woven_rest.extend(proj_unit(wt, dst, 1, nch))

        # V: out[n, c] = xT[:, ntile].T @ wv   -> [128 n, 256 c]
        def v_unit(nt):
            state = {}

            def half(h):
                if h == 0:
                    state["ps"] = spool.tile([P, HL * DH], f32, tag="sp", name="psv")
                ps = state["ps"]
                for kt in range(4 * h, 4 * h + 4):
                    nc.tensor.matmul(
                        ps[:],
                        xT[:, nt // 4, kt, (nt % 4) * P : (nt % 4 + 1) * P],
                        wv[:, :, kt, :],
                        start=(kt == 0),
                        stop=(kt == KT - 1),
                    )
                if h == 1:
                    # scatter the 4 heads' 64 cols into the [NT, HL, 65] layout
                    nc.vector.tensor_copy(
                        v[:, nt, :, 0:DH],
                        ps[:].rearrange("p (h d) -> p h d", h=HL),
                    )

            return [lambda: half(0), lambda: half(1)]

        v_units = [full_unit(v_unit(nt)) for nt in range(NT)]

        # ---- attention ----
        # 8 blocks of 16 periods (one per (hp, ic)).  ACT runs one
        # [128, 1024] exp per period back-to-back; PE emits scores two
        # periods ahead (spool rotation) plus woven projection work; PV runs
        # as dense 8-matmul bursts every 4 periods (no exp-latency exposure).
        # Block 0 weaves the V projection (PV bursts shifted late until V is
        # ready); blocks 1+ weave the remaining q/k projections.
        blocks = [(hp, ic) for hp in range(2) for ic in range(NCH)]
        ats = {}
        opairs = {}
        sp_ahead = {}

        def emit_scores(b, jb):
            hp, ic = blocks[b]
            i0, j0 = ic * 512, jb * P
            sp = spool.tile([P, 1024], f32, tag="sp", name="sp")
            nc.tensor.matmul(
                sp[:, 0:512],
                kT[hp][0:DH, j0 : j0 + P],
                qT[hp][0:DH, i0 : i0 + 512],
                start=True, stop=True, tile_position=(0, 0),
            )
            nc.tensor.matmul(
                sp[:, 512:1024],
                kT[hp][DH:P, j0 : j0 + P],
                qT[hp][DH:P, i0 : i0 + 512],
                start=True, stop=True, tile_position=(64, 0),
            )
            return sp

        def emit_exp(b, jb, sp):
            at = apool.tile([P, 1024], bf16, tag="at", name="at")
            nc.scalar.activation(at[:], sp[:], mybir.ActivationFunctionType.Exp)
            ats[(b, jb)] = at

        def fetch_scores(b, jb):
            key = (b, jb)
            if key in sp_ahead:
                return sp_ahead.pop(key)
            return emit_scores(b, jb)

        def emit_pv(b, jbs, writeback=False):
            """PV matmuls of block b for the given j-blocks (dense burst);
            with writeback, each head's copy+DMA follows its last matmul so
            it overlaps the other head's PV."""
            hp, ic = blocks[b]
            if b not in opairs:
                opairs[b] = (
                    opool.tile([DH + 1, 512], f32, tag="oA", name="oA"),
                    opool.tile([DH + 1, 512], f32, tag="oB", name="oB"),
                )
            oA, oB = opairs[b]
            i0 = ic * 512
            for col, o in ((0, oA), (1, oB)):
                for jb in jbs:
                    nc.tensor.matmul(
                        o[:],
                        v[:, jb, 2 * hp + col, :],
                        ats[(b, jb)][:, 512 * col : 512 * col + 512],
                        start=(jb == 0), stop=(jb == NT - 1),
                    )
                if writeback:
                    os = copool.tile([DH + 1, 1, 512], bf16, tag="os", name="os")
                    nc.vector.tensor_copy(os[:, 0, :], o[:])
                    nc.sync.dma_start(
                        out=out_r[:, 2 * hp + col : 2 * hp + col + 1, i0 : i0 + 512],
                        in_=os[:],
                    )
            for jb in jbs:
                del ats[(b, jb)]

        def emit_pv_quarter(b, q):
            emit_pv(b, list(range(4 * q, 4 * q + 4)), writeback=(q == 3))

        LA = 2  # scores lookahead depth
        nblocks = len(blocks)
        # prime the pipeline: scores(0,0) right after the interleaved narrow
        # chains so the first exp starts as soon as chunk 0 has landed
        upfront_chains()
        sp_ahead[(0, 0)] = emit_scores(0, 0)
        upfront_krest(P, 2 * P)
        sp_ahead[(0, 1)] = emit_scores(0, 1)
        upfront_krest(2 * P, 512)
        for b in range(nblocks):
            for jb in range(NT):
                emit_exp(b, jb, fetch_scores(b, jb))
                la = jb + LA
                if la < NT:
                    if (b, la) not in sp_ahead:
                        sp_ahead[(b, la)] = emit_scores(b, la)
                elif b + 1 < nblocks:
                    sp_ahead[(b + 1, la - NT)] = emit_scores(b + 1, la - NT)
                if jb == NT - 1 and b + 1 < nblocks:
                    # boundary prefetch into the idle third spool slot: gives
                    # ACT a 3rd exp of cover across the 16-matmul PV burst
                    sp_ahead[(b + 1, LA)] = emit_scores(b + 1, LA)
                # woven PE filler: 1 unit/period in block 0 plus 2 V units
                # once the k/q chunks are done; then 1 unit per 6 periods,
                # with the last two q-hp1 chunks deferred to blocks 5-6
                # where the PE has slack
                p = b * NT + jb
                if b == 0:
                    if woven:
                        woven.pop(0)()
                    for _ in range(2):
                        if not woven and v_units:
                            v_units.pop(0)()
                elif woven_rest and (p - NT) % 6 in (5, 0) and 20 < p < 65:
                    woven_rest.pop(0)()
                elif woven_rest and p in (80, 81, 88, 89):
                    woven_rest.pop(0)()
                # PV bursts (block 0's deferred until the woven V is ready);
                # each block's final quarter is split into two half-bursts so
                # the block boundary doesn't pile 8 matmuls after the last exp
                if b == 0:
                    if jb == 8:
                        emit_pv_quarter(0, 0)
                    elif jb == 10:
                        emit_pv_quarter(0, 1)
                    elif jb == 12:
                        emit_pv_quarter(0, 2)
                elif jb in (4, 8, 12):
                    emit_pv_quarter(b, jb // 4 - 1)
                if jb == NT - 2:
                    emit_pv(b, [12, 13, 14])
                elif jb == NT - 1:
                    while v_units:
                        v_units.pop(0)()
                    emit_pv(b, [15], writeback=True)

    nc.compile()
    return nc


def _get_nc():
    if "nc" not in _CACHE:
        _CACHE["nc"] = _build_nc()
    return _CACHE["nc"]


def _prepare_in_maps(x, w_qkv):
    bf = ml_dtypes.bfloat16
    x = np.asarray(x, dtype=np.float32)
    w = np.asarray(w_qkv, dtype=np.float32)
    scale = DH ** -0.5
    in_maps = []
    xT_b = [
        np.ascontiguousarray(
            x[b].T.reshape(KT, P, NCH, 512).transpose(1, 2, 0, 3).reshape(P, KT * N)
        ).astype(bf)
        for b in range(B)
    ]
    def pack_w(w_slice):
        # [1024, 256] -> [p, hp, kt, c] -> [128, 2048]
        t = w_slice.reshape(KT, P, 2, P).transpose(1, 2, 0, 3).reshape(P, 2 * KT * P)
        return np.ascontiguousarray(t).astype(bf)

    for c in range(8):
        b, hg = divmod(c, 4)
        cs = slice(hg * HL * DH, (hg + 1) * HL * DH)
        in_maps.append(
            {
                "xT": xT_b[b],
                "wq": pack_w(w[:, cs] * scale),
                "wk": pack_w(w[:, 1024:2048][:, cs]),
                "wv": pack_w(w[:, 2048:3072][:, cs]),
            }
        )
    return in_maps


def _assemble(outs):
    full = np.empty((B, N, HEADS * DH), dtype=np.float32)
    for c in range(8):
        b, hg = divmod(c, 4)
        o = np.asarray(outs[c], dtype=np.float32).reshape(HL, DH + 1, N)
        norm = o[:, :DH, :] / o[:, DH : DH + 1, :]  # [hl, d, n]
        full[b, :, hg * HL * DH : (hg + 1) * HL * DH] = norm.transpose(2, 0, 1).reshape(
            N, HL * DH
        )
    return full


def kernel(x, w_qkv):
    global LAST_RESULTS
    from concourse.bass_utils import run_bass_kernel_spmd

    nc = _get_nc()
    in_maps = _prepare_in_maps(x, w_qkv)
    last_err = None
    for _ in range(3):  # the runtime occasionally throws a transient device error
        try:
            res = run_bass_kernel_spmd(
                nc,
                in_maps,
                core_ids=list(range(8)),
                trace=TRACE,
                trace_cores=[0] if TRACE else None,
            )
            break
        except Exception as e:
            last_err = e
    else:
        raise last_err
    LAST_RESULTS = res
    return _assemble([r["out"] for r in res.results])

